# revision 77
# baseline (speedup 1.0000x reference)
"""Paged-KV varlen causal GQA attention for Trainium2, sharded over 8 NeuronCores.

Problem (hardcoded from spec): T=4096 tokens, 16 q heads / 8 kv heads, head_dim=64,
scale=0.125. k/v are scattered into paged caches via slot_mapping, read back, and
causal varlen attention (segments from cu_seqlens) is computed.

Sharding: tensor-parallel over kv heads -- core h gets kv head h and its 2 GQA
query heads. slot_mapping / cu_seqlens handled on host (index math only).

Device kernel (per core): TWO independent score-tile streams (so each exp
engine paces its own PSUM double-buffer):
  A stream [128,1024] f32 x2 bufs:  se = exp(0.125*sp) on ScalarE
  D stream [128, 512] f32 x2 bufs:  se = 2^(0.125*sp*log2e) on VectorE via the
      Schraudolph int16 bit trick written straight into bf16 (~3% weight err;
      carries the diagonal blocks except query-chunk 0, plus enough late
      non-diagonal stripes to balance ScalarE vs VectorE)
  sp[keys, queries] = kT.T @ qT          (PE, per piece, bank-split)
  diagonal blocks: se *= trimask         (GpSimd mid-stream, VectorE at the tail)
  o[q, 0:65] += se[:,qc].T @ [v | 1]     (PE with se as STATIONARY: output lands
                                          query-major, col 64 = softmax denom --
                                          no transposes / PSUM->SBUF copies;
                                          accumulation start/stop flags are per
                                          PSUM bank, set by the first/last PV)
  osb[q, d] = o[q, d] * 1/o[q, 64]       (VectorE reciprocal + broadcast mul)
"""

import os
from contextlib import ExitStack
from math import ceil

import numpy as np
import ml_dtypes

import concourse.bass as bass
import concourse.mybir as mybir
import concourse.tile as tile
from concourse import bacc
from concourse.bass_utils import run_bass_kernel_spmd

NKV = 8
G = 2
D = 64
SCALE = 0.125

TILE_A = 1024  # ScalarE score tile width (2 PSUM banks of f32, 2 buffers)
TILE_D = 512   # DVE exp2 score tile width (1 PSUM bank, 2 buffers)
BANK = 512     # f32 cols per PSUM bank
W0Q = 1024     # warmup tensor covers kT[0:128] + qT[0:1024] of segment 0

# test.py pokes these for profiling
TRACE = bool(int(os.environ.get("KERNEL_TRACE", "0")))
LAST_RESULT = None

_PROGRAM_CACHE = {}


def _plan(segments):
    """Pack the per-(segment, head) score work into TWO independent tile
    streams, interleaved in one creation-order list:
      kind 'a': ScalarE-exp tiles, [128, <=1024] f32 (2 PSUM banks x 2 bufs)
      kind 'd': DVE-exp2 tiles,    [128, <=512]  f32 (1 PSUM bank  x 2 bufs)
    Each engine paces its own stream, so an sp slot is freed by exactly one
    reader and the pipelines decouple.

    Tile dicts: kind, sz, qk, ts, masks, pmask, pv, norms, dmas.
    """
    tiles = []
    cur = {"a": None, "d": None}
    acap = [384, 640]  # warmup caps for the first A tiles, then TILE_A
    chains = {}  # (h, si, qc) -> [ [tile_idx, col, cw, ktg, kp], ... ]

    def new_tile(kind):
        t = {
            "kind": kind, "idx": len(tiles), "sz": 0, "qk": [], "ts": [],
            "masks": [], "pmask": [], "pv": [], "norms": [], "dmas": [],
        }
        tiles.append(t)
        cur[kind] = t
        return t

    def close(kind):
        cur[kind] = None

    def cap(kind):
        if kind == "d":
            return TILE_D
        na = sum(1 for t in tiles if t["kind"] == "a")
        if cur["a"] is not None:
            na -= 1
        return acap[na] if na < len(acap) else TILE_A

    # global key-tile counter (vt layout is per-segment tiled on host)
    seg_ktg0 = []
    nkt_tot = 0
    for (s0, s1) in segments:
        seg_ktg0.append(nkt_tot)
        nkt_tot += ceil((s1 - s0) / 128)

    def place_span(kind, h, si, kt, qa, qb, w0_ok):
        """Place one query span of key tile kt into the `kind` stream,
        splitting at tile caps and PSUM bank edges; register PV chunks.
        Returns [(tile, col, ncols)] placements."""
        s0, s1 = segments[si]
        L = s1 - s0
        klo = 128 * kt
        kp = min(128, L - klo)
        ktg = seg_ktg0[si] + kt
        placements = []
        q = qa
        while q < qb:
            if cur[kind] is None or cur[kind]["sz"] >= cap(kind):
                new_tile(kind)
            t = cur[kind]
            room = cap(kind) - t["sz"]
            take = min(qb - q, room)
            col = t["sz"]
            placements.append((t, col, take))
            qq = q
            while qq < q + take:
                c = col + (qq - q)
                lim = qq + min(q + take - qq, BANK - (c % BANK))
                use_w0 = w0_ok and qq < W0Q
                if use_w0 and lim > W0Q:
                    lim = W0Q
                t["qk"].append((h, s0, klo, kp, qq, lim, c, use_w0))
                qq = lim
            for qs in range(q, q + take, 128):
                qc = qs // 128
                cw = min(128, qb - qs, q + take - qs)
                chains.setdefault((h, si, qc), []).append(
                    [t["idx"], col + (qs - q), cw, ktg, kp]
                )
            t["sz"] += take
            if t["sz"] % 128:
                t["sz"] += 128 - t["sz"] % 128
            q += take
        return placements

    def place_diag(h, si, kts, act, w0_ok=False, fast_mask=False):
        """Diagonal chunks: kt0 (act=True) goes on the A stream with a
        mask-multiply (pmask); others go on the D stream (DVE exp2) with
        the causal mask applied by Pool (or DVE when fast_mask)."""
        s0, s1 = segments[si]
        L = s1 - s0
        for kt in kts:
            klo = 128 * kt
            kp = min(128, L - klo)
            pl = place_span("a" if act else "d", h, si, kt, klo, klo + kp, w0_ok)
            for (t, col, n) in pl:
                if act:
                    t["pmask"].append((col, fast_mask))
                else:
                    t["ts"].append((col, n))
                    t["masks"].append((col, n // 128, fast_mask))

    def take_nd(kind, h, si, ndq, ncols, w0_ok=False):
        """Consume ncols of non-diagonal pieces into the given stream; D-
        stream cols get ts entries (DVE exp2, no mask needed)."""
        left = ncols
        while left > 0:
            kt, qa, qb = ndq[0]
            take = min(qb - qa, left)
            pl = place_span(kind, h, si, kt, qa, qa + take, w0_ok)
            if kind == "d":
                for (t, col, n) in pl:
                    t["ts"].append((col, n))
            if qa + take == qb:
                ndq.pop(0)
            else:
                ndq[0][1] = qa + take
            left -= take

    nseg = len(segments)
    for si, (s0, s1) in enumerate(segments):
        L = s1 - s0
        nkt = ceil(L / 128)
        nqc = ceil(L / 128)
        for h in range(G):
            first_sh = si == 0 and h == 0 and L >= 1024
            last_sh = (
                si == nseg - 1 and h == G - 1 and nqc >= 8 and L % 128 == 0
            )
            nd = [kt for kt in range(nkt - 1) if 128 * (kt + 1) < L]
            fast = L == 1024 and nkt == 8
            if fast and not last_sh:
                w0_ok = first_sh
                ndq = [[kt, 128 * (kt + 1), L] for kt in nd]
                # route the last 640 nd cols (late stripes of kt4..6, well
                # diluted query chunks) through the DVE stream; with the kt0
                # diagonal on the A stream both streams then hold an exact
                # number of tiles per (seg, head), which keeps oh-bank norms
                # strictly before the next tenant's first PV in program order
                extras = [[4, 768, 1024], [5, 768, 1024], [6, 896, 1024]]
                ndq[4][2] = 768
                ndq[5][2] = 768
                ndq = [p for p in ndq if p[1] < p[2] and p[0] != 6]
                take_nd("a", h, si, ndq, 896, w0_ok)
                place_diag(h, si, [0], True, w0_ok)
                place_diag(h, si, [1, 2, 3, 4], False, w0_ok)
                take_nd("a", h, si, ndq, 1024, w0_ok)
                place_diag(h, si, [5, 6, 7], False, w0_ok)
                take_nd("d", h, si, extras, 640, w0_ok)
                take_nd("a", h, si, ndq, 1024, w0_ok)
                close("a")
                close("d")
            elif fast and last_sh:
                # tail: qchunk 7's pieces and the final diagonal chunk come
                # last, in their own small tiles, for a short closing chain
                ndqa = [[kt, 128 * (kt + 1), L - 128] for kt in nd]
                ndqb = [[kt, L - 128, L] for kt in nd]
                extras = [[4, 640, 896], [5, 768, 896]]
                ndqa[4][2] = 640
                ndqa[5] = [5, 768, 768]
                ndqa = [p for p in ndqa if p[1] < p[2]]
                take_nd("a", h, si, ndqa, 1024)
                place_diag(h, si, [1, 2, 3], False, fast_mask=True)
                take_nd("a", h, si, ndqa, 1024)
                place_diag(h, si, [4, 5, 6], False, fast_mask=True)
                take_nd("d", h, si, extras, 384)
                take_nd("a", h, si, ndqa, 256)
                place_diag(h, si, [0], True, fast_mask=True)
                close("a")
                take_nd("a", h, si, ndqb, 896)
                place_diag(h, si, [7], True, fast_mask=True)
                close("a")
                close("d")
            else:
                # generic fallback: everything on the A stream, diag last,
                # masks on Pool
                for kt in nd:
                    place_span("a", h, si, kt, 128 * (kt + 1), L, first_sh)
                place_diag(h, si, [0], True, first_sh)
                place_diag(h, si, list(range(1, nkt)), False, first_sh)

    # coalesce adjacent ts / mask ranges per tile
    for t in tiles:
        for key in ("ts",):
            t[key].sort()
            merged = []
            for (c0, w) in t[key]:
                if merged and merged[-1][0] + merged[-1][1] == c0:
                    merged[-1][1] += w
                else:
                    merged.append([c0, w])
            t[key] = [tuple(m) for m in merged]
        t["masks"].sort()
        merged = []
        for (c0, n, fm) in t["masks"]:
            if merged and merged[-1][0] + 128 * merged[-1][1] == c0 and merged[-1][2] == fm:
                merged[-1][1] += n
            else:
                merged.append([c0, n, fm])
        t["masks"] = [tuple(m) for m in merged]

    # qchunk -> oh-bank mapping: 4 chunks per PSUM bank, except the last
    # (segment, head) where the final chunk gets its own bank so its
    # accumulation group closes independently
    def oh_banks(h, si):
        L = segments[si][1] - segments[si][0]
        nqc = ceil(L / 128)
        if si == nseg - 1 and h == G - 1 and nqc == 8:
            return [[0, 1, 2], [3, 4, 5, 6], [7]]
        return [
            list(range(4 * f, min(4 * f + 4, nqc))) for f in range(ceil(nqc / 4))
        ]

    qc_bank = {}  # (h, si, qc) -> (f, fbase)
    for si in range(nseg):
        for h in range(G):
            for f, qcs in enumerate(oh_banks(h, si)):
                for qc in qcs:
                    qc_bank[(h, si, qc)] = (f, qcs[0])

    # start/stop flags are PER PSUM BANK (zero region), not per qchunk chain:
    # start_tensor_calc pends-zero the whole 2KB bank, so only the
    # chronologically first matmul into an oh tile may set it, and only the
    # last sets stop. Per-byte lazy zeroing makes each chain's first write a
    # plain store and later writes accumulates, regardless of interleaving.
    oh_groups = {}
    for (h, si, qc), ch in chains.items():
        for e in ch:
            oh_groups.setdefault((h, si, qc_bank[(h, si, qc)][0]), []).append(e)
    for grp in oh_groups.values():
        grp.sort(key=lambda e: (e[0], e[1]))
        for e in grp:
            e.append(e is grp[0])
            e.append(e is grp[-1])
    for (h, si, qc), ch in chains.items():
        for e in ch:
            tiles[e[0]]["pv"].append((h, si, qc, e[1], e[2], e[3], e[4], e[5], e[6]))

    # order pv entries in each tile by emission col to keep per-chain order
    for t in tiles:
        t["pv"].sort(key=lambda p: (p[3], p[0], p[1], p[2]))

    # norm groups (one per oh bank): due after max last tile over the chains
    for si in range(nseg):
        norm_due = {}
        for h in range(G):
            for f, gq in enumerate(oh_banks(h, si)):
                last = max(chains[(h, si, qc)][-1][0] for qc in gq)
                tiles[last]["norms"].append((h, si, f, gq[0], len(gq)))
                for qc in gq:
                    norm_due[(h, qc)] = last
        row_groups = oh_banks(G - 1, si)
        if len(row_groups) > 2:
            for gq in row_groups:
                due = max(norm_due[(h, qc)] for h in range(G) for qc in gq)
                tiles[due]["dmas"].append((si, gq[0], gq[-1] + 1))
        else:
            due = max(norm_due.values())
            nqc = ceil((segments[si][1] - segments[si][0]) / 128)
            tiles[due]["dmas"].append((si, 0, nqc))

    return tiles, nkt_tot, qc_bank


def _build_program(T, segments):
    f32 = mybir.dt.float32
    bf16 = mybir.dt.bfloat16

    tiles, nkt_tot, qc_bank = _plan(segments)
    L0 = segments[0][1] - segments[0][0]
    use_w0 = L0 >= 1024

    nc = bacc.Bacc(
        "TRN2",
        target_bir_lowering=False,
        debug=False,
        enable_asserts=False,
        num_devices=8,
    )
    qT_d = nc.dram_tensor("qT", [128, T], bf16, kind="ExternalInput").ap()
    kT_d = nc.dram_tensor("kT", [64, T], bf16, kind="ExternalInput").ap()
    vt_d = nc.dram_tensor("vt", [128, nkt_tot * 65], bf16, kind="ExternalInput").ap()
    if use_w0:
        w0_d = nc.dram_tensor("w0", [64, 128 + W0Q], bf16, kind="ExternalInput").ap()
    o_d = nc.dram_tensor("o", [T, G * D], f32, kind="ExternalOutput").ap()

    with tile.TileContext(nc) as tc, ExitStack() as ctx:
        const = ctx.enter_context(tc.tile_pool(name="const", bufs=1))
        inpool = ctx.enter_context(tc.tile_pool(name="inp", bufs=1))
        sepool = ctx.enter_context(tc.tile_pool(name="se", bufs=10))
        sdpool = ctx.enter_context(tc.tile_pool(name="sed", bufs=10))
        opool = ctx.enter_context(tc.tile_pool(name="osb", bufs=3))
        rpool = ctx.enter_context(tc.tile_pool(name="rcp", bufs=6))
        ps_a = ctx.enter_context(tc.tile_pool(name="ps_a", bufs=2, space="PSUM"))
        ps_d = ctx.enter_context(tc.tile_pool(name="ps_d", bufs=2, space="PSUM"))
        ps_o = ctx.enter_context(tc.tile_pool(name="ps_o", bufs=2, space="PSUM"))

        # trimask[p, c] = 1 if c >= p else 0 (valid = query col >= key partition)
        trimask = const.tile([128, 128], bf16)
        nc.gpsimd.memset(trimask, 0.0)
        nc.gpsimd.affine_select(
            out=trimask,
            in_=trimask,
            compare_op=mybir.AluOpType.is_gt,
            fill=1.0,
            base=0,
            pattern=[[-1, 128]],
            channel_multiplier=1,
        )

        qT = inpool.tile([128, T], bf16)
        kT = inpool.tile([128, T], bf16)
        vt = inpool.tile([128, nkt_tot * 65], bf16)
        vtv = vt.rearrange("p (n k) -> p n k", k=65)
        if use_w0:
            w0 = inpool.tile([64, 128 + W0Q], bf16, padded_shape=[128, 128 + W0Q])

        # ---- input DMAs: all issued up front ----
        # kT halves are loaded separately (h0 rows 0:64 first) so head 0's QK
        # stream unblocks as early as possible; w0 covers the whole first
        # (seg0, h0) non-diagonal stream so tiles 0..3 only depend on 1-2 DMAs
        s00, s01 = segments[0]
        kg = 0
        if use_w0:
            nc.sync.dma_start(w0, w0_d)
        first = True
        for (s0, s1) in segments:
            nkt = ceil((s1 - s0) / 128)
            nc.sync.dma_start(kT[0:64, s0:s1], kT_d[:, s0:s1])
            if first and use_w0:
                nc.sync.dma_start(qT[:, s0:s1], qT_d[:, s0:s1])
                nc.sync.dma_start(
                    vt[:, kg * 65 : (kg + nkt) * 65],
                    vt_d[:, kg * 65 : (kg + nkt) * 65],
                )
            else:
                nc.sync.dma_start(qT[:, s0:s1], qT_d[:, s0:s1])
                nc.sync.dma_start(
                    vt[:, kg * 65 : (kg + nkt) * 65],
                    vt_d[:, kg * 65 : (kg + nkt) * 65],
                )
            nc.sync.dma_start(kT[64:128, s0:s1], kT_d[:, s0:s1])
            kg += nkt
            first = False

        # ---- main stream ----
        se_tiles = {}
        oh_tiles = {}
        osb_tiles = {}

        def get_oh(h, si, f):
            key = (h, si, f)
            if key not in oh_tiles:
                oh_tiles[key] = ps_o.tile(
                    [128, 512], f32, tag="oh", name=f"oh_{h}_{si}_{f}"
                )
            return oh_tiles[key]

        def get_osb(si):
            if si not in osb_tiles:
                nqc = ceil((segments[si][1] - segments[si][0]) / 128)
                osb_tiles[si] = opool.tile(
                    [128, nqc * 128], f32, tag="osb", name=f"osb_{si}"
                )
            return osb_tiles[si]

        def emit_qk(g, sp):
            # diagonal subpieces first: the DVE exp2 pass only needs those,
            # so it can start while the rest of the tile's QKs still run
            dcols = set()
            for (c0, w) in tiles[g]["ts"]:
                dcols.update(range(c0, c0 + w, 128))
            tiles[g]["qk"].sort(key=lambda p: (p[6] not in dcols, p[6]))
            for (h, s0, klo, kp, qa, qb, col, from_w0) in tiles[g]["qk"]:
                lhsT = (
                    w0[0:64, 0:kp]
                    if (from_w0 and klo == 0)
                    else kT[64 * h : 64 * h + 64, s0 + klo : s0 + klo + kp]
                )
                rhs = (
                    w0[0:64, 128 + qa : 128 + qb]
                    if from_w0
                    else qT[64 * h : 64 * h + 64, s0 + qa : s0 + qb]
                )
                nc.tensor.matmul(
                    sp[:kp, col : col + qb - qa],
                    lhsT,
                    rhs,
                    start=True,
                    stop=True,
                    tile_position=(64 * h, 0),
                )

        def emit_post(g):
            t = tiles[g]
            se = se_tiles.pop(g)
            for (h, si, qc, col, cw, ktg, kp, start, stop) in t["pv"]:
                f, fbase = qc_bank[(h, si, qc)]
                oh = get_oh(h, si, f)
                lq = qc - fbase
                nc.tensor.matmul(
                    oh[:cw, 128 * lq : 128 * lq + 65],
                    se[:kp, col : col + cw],
                    vtv[:kp, ktg, :],
                    start=start,
                    stop=stop,
                )
            for (h, si, f, qc0, nq) in t["norms"]:
                oh = oh_tiles[(h, si, f)]
                osb = get_osb(si)
                lq = 0  # bank-local offset: each norm group is a whole oh tile
                uniq = f"{h}_{si}_{qc0}"
                rcp = rpool.tile([128, 8], f32, tag="rcp", name=f"rcp_{uniq}")
                nc.vector.reciprocal(
                    rcp[:, :nq],
                    oh[:, 128 * lq + 64 : 128 * (lq + nq - 1) + 65 : 128],
                )
                ohv = oh.rearrange("p (c k) -> p c k", k=128)[:, lq : lq + nq, 0:D]
                rv = rcp[:, :nq].rearrange("p (c k) -> p c k", k=1)
                rv, ohv = bass.broadcast_tensor_aps(rv, ohv)
                nc.vector.tensor_mul(
                    osb.rearrange("p (c k) -> p c k", k=128)[
                        :, qc0 : qc0 + nq, D * h : D * h + D
                    ],
                    ohv,
                    rv,
                )
            for (si, c0, c1) in t["dmas"]:
                s0 = segments[si][0]
                osb = osb_tiles[si]
                L = segments[si][1] - s0
                r0, r1 = 128 * c0, min(128 * c1, L)
                nfc = (r1 - r0) // 128
                if nfc:
                    nc.sync.dma_start(
                        o_d[s0 + r0 : s0 + r0 + nfc * 128, :].rearrange(
                            "(c p) k -> p c k", p=128
                        ),
                        osb.rearrange("p (c k) -> p c k", k=128)[:, c0 : c0 + nfc, :],
                    )
                if (r1 - r0) % 128:
                    rr = r0 + nfc * 128
                    nc.sync.dma_start(
                        o_d[s0 + rr : s0 + r1, :],
                        osb[: r1 - rr, 128 * (c0 + nfc) : 128 * (c0 + nfc) + G * D],
                    )

        # Schraudolph exp2: exp(SCALE*s) = 2^(SCALE*s*log2e), assembled as
        # an int16 bit pattern that IS the bf16 weight: i16 = trunc(t*2^7 +
        # ((127<<7) - C)). ~3% max weight error; used on diagonal blocks
        # (query chunk 0 stays exact on ScalarE) and a slice of non-diagonal
        # cols to balance the ScalarE and Vector engines.
        LOG2E = 1.4426950408889634
        A_TS = float(SCALE * LOG2E * 128.0)
        B_TS = float((127 << 7) - 2.8)
        i16dt = mybir.dt.int16

        first_a = next(i for i, t in enumerate(tiles) if t["kind"] == "a")
        for g in range(len(tiles)):
            t = tiles[g]
            sz = t["sz"]
            if t["kind"] == "a" and g == first_a and sz <= BANK:
                # the oh banks sit idle until the first PVs (~7us in); borrow
                # one for the first score tile so the third A tile's QK does
                # not wait on the warmup exps to free a ps_a slot
                sp = ps_o.tile([128, 512], f32, tag="oh", name=f"spw_{g}")
                se = sepool.tile([128, TILE_A], bf16, tag="se", name=f"se_{g}")
            elif t["kind"] == "a":
                sp = ps_a.tile([128, TILE_A], f32, tag="spa", name=f"spa_{g}")
                se = sepool.tile([128, TILE_A], bf16, tag="se", name=f"se_{g}")
            else:
                sp = ps_d.tile([128, TILE_D], f32, tag="spd", name=f"spd_{g}")
                se = sdpool.tile([128, TILE_D], bf16, tag="sed", name=f"sed_{g}")
            emit_qk(g, sp)
            se_tiles[g] = se
            # DVE exp2 bit trick straight into bf16: i16 = t*2^7 + bias is
            # the bit pattern of ~2^t in bf16 (one tensor_scalar, no copy)
            for (c0, w) in t["ts"]:
                nc.vector.tensor_scalar(
                    se[:, c0 : c0 + w].bitcast(i16dt),
                    sp[:, c0 : c0 + w],
                    A_TS,
                    B_TS,
                    mybir.AluOpType.mult,
                    mybir.AluOpType.add,
                )
            # causal mask for diagonal chunks (cheap bf16 multiply; Pool for
            # mid-stream tiles, DVE for the latency-critical closing tiles)
            for (c0, n, fm) in t["masks"]:
                sev = se[:, c0 : c0 + 128 * n].rearrange("p (c k) -> p c k", k=128)
                triv = trimask.rearrange("p (c k) -> p c k", k=128)
                triv, sev = bass.broadcast_tensor_aps(triv, sev)
                eng = nc.vector if fm else nc.gpsimd
                eng.tensor_mul(
                    se[:, c0 : c0 + 128 * n].rearrange("p (c k) -> p c k", k=128),
                    sev,
                    triv,
                )
            if g >= 7:
                emit_post(g - 7)
            # ScalarE exp over the complement of the DVE ranges
            pos = 0
            acts = []
            for (c0, w) in t["ts"]:
                if c0 > pos:
                    acts.append((pos, c0))
                pos = c0 + w
            if pos < sz:
                acts.append((pos, sz))
            for (a, b) in acts:
                nc.scalar.activation(
                    se[:, a:b],
                    sp[:, a:b],
                    mybir.ActivationFunctionType.Exp,
                    scale=SCALE,
                )
            # mask for ScalarE-path diagonal chunks (query chunk 0)
            for (c, fm) in t["pmask"]:
                eng = nc.vector if fm else nc.gpsimd
                eng.tensor_mul(se[:, c : c + 128], se[:, c : c + 128], trimask)
        for gg in range(max(0, len(tiles) - 7), len(tiles)):
            emit_post(gg)

    nc.compile()
    return nc


def _segments_from_cu(cu_seqlens, T):
    edges = sorted(set([0, T] + [int(c) for c in cu_seqlens if 0 < int(c) < T]))
    return [(edges[i], edges[i + 1]) for i in range(len(edges) - 1)]


def kernel(q, k, v, k_cache, v_cache, slot_mapping, cu_seqlens):
    global LAST_RESULT
    T = q.shape[0]
    nslots = k_cache.shape[0]

    # Emulate scatter-then-gather through the paged cache: for duplicate slots
    # the last writer wins, so token i reads back k[lastw[slot[i]]].
    slot = np.asarray(slot_mapping, dtype=np.int64)
    lastw = np.zeros(nslots, dtype=np.int64)
    lastw[slot] = np.arange(T)
    lw = lastw[slot]
    k_eff = np.asarray(k)[lw]
    v_eff = np.asarray(v)[lw]

    segments = _segments_from_cu(np.asarray(cu_seqlens), T)
    key = (T, tuple(segments))
    if key not in _PROGRAM_CACHE:
        _PROGRAM_CACHE[key] = _build_program(T, segments)
    nc = _PROGRAM_CACHE[key]

    bf = ml_dtypes.bfloat16
    qh = np.ascontiguousarray(
        np.asarray(q).reshape(T, NKV * G, D).transpose(1, 2, 0)
    ).astype(bf)  # [16, 64, T]
    kh = np.ascontiguousarray(k_eff.reshape(T, NKV, D).transpose(1, 2, 0)).astype(bf)
    vh = v_eff.reshape(T, NKV, D).astype(bf)  # [T, 8, 64]

    # vt: per-segment 128-row tiling of v rows, with a ones column at k=64
    nkt_tot = sum(ceil((s1 - s0) / 128) for (s0, s1) in segments)
    L0 = segments[0][1] - segments[0][0]
    use_w0 = L0 >= 1024

    in_maps = []
    for h in range(NKV):
        qT = np.ascontiguousarray(qh[2 * h : 2 * h + 2].reshape(128, T))
        kT = np.ascontiguousarray(kh[h])  # [64, T]
        vt = np.zeros((128, nkt_tot, 65), dtype=bf)
        kg = 0
        for (s0, s1) in segments:
            L = s1 - s0
            for kt in range(ceil(L / 128)):
                klo = s0 + 128 * kt
                kp = min(128, s1 - klo)
                vt[:kp, kg, :D] = vh[klo : klo + kp, h, :]
                vt[:, kg, D] = 1.0
                kg += 1
        m = {
            "qT": qT,
            "kT": kT,
            "vt": np.ascontiguousarray(vt.reshape(128, nkt_tot * 65)),
        }
        if use_w0:
            s00 = segments[0][0]
            w0 = np.concatenate(
                [kT[:, s00 : s00 + 128], qT[0:64, s00 : s00 + W0Q]], axis=1
            )
            m["w0"] = np.ascontiguousarray(w0)
        in_maps.append(m)

    res = run_bass_kernel_spmd(nc, in_maps, core_ids=list(range(8)), trace=TRACE)
    LAST_RESULT = res

    out = np.empty((T, NKV * G * D), dtype=np.float32)
    ov = out.reshape(T, NKV, G * D)
    for h in range(NKV):
        ov[:, h, :] = res.results[h]["o"]
    return out
